# revision 13
# baseline (speedup 1.0000x reference)
"""Trainium2 Bass kernel for per-time-slice spatial self-attention + 1x1 conv.

Math per (b, t) slice (16 slices total):
    x      = x_in[b, :, t]          reshaped [C=64, P=2304]
    theta  = theta_w[t] @ x         [32, P]
    phi    = phi_w[t] @ x           [32, P]
    S      = theta.T @ phi / sqrt(32)          [P, P]
    A      = softmax(S, axis=-1)
    f      = x @ A.T  (f[c,p] = sum_q A[p,q] x[c,q])
    out    = out_w @ f + x

Sharding: the 16 slices are independent -> 2 slices per NeuronCore, no
collectives. Host precomputes the cheap channel projections (u, v below)
and packs layouts; the device runs the O(P^2) attention core:

  per p-chunk (4x512 + 256) accumulating in PSUM over 18 q-tiles of 128:
    scoresT[q, p] = sum_c x[c, q] u[c, p]         (PE, K=128 Gram trick:
        u = (phi_w^T theta_w) x host-side, x zero-padded to 128 rows as
        the stationary operand so junk rhs rows are killed by zero
        weights. HW-measured (microbench.py): K=128 matmuls stream at
        ~167ns for N=512 while the K=32 theta/phi form measures
        300-450ns - partial-K matmuls miss the fast stream path.)
    E ~= exp(scoresT / sqrt(32))                  (VectorE, one fused
        tensor_scalar: i16 = int16(s * 128*log2e*scale + 16256); the i16
        bit pattern bitcast as bf16 IS the Schraudolph base-2 exp
        approximation; the softmax normalization cancels the bulk of the
        correlated approximation error: end-to-end max rel err ~2.4e-3.)
    val[m, p] += vte[q, m]^T E[q, p]              (PE, m: 64 v-channels + ones
                                                   column -> softmax denom)
  epilogue: one ACT copy val -> SBUF per chunk; one [65, 2304] DMA per
  slice (output DMAs ride the qAct HWDGE ring, inputs ride qSP, so the
  two streams don't serialize). The final normalization (row 64 divide)
  and the +x residual are done on the host after the gather.

exp skips max-subtraction (scores ~ N(0,1), max |s| ~ 12.6; safe in this
fixed-point window: t = 128*z + 16256 stays within int16 for |z| < 127).
"""

import os
import sys

for _p in ("/opt/trn_rl_repo", "/root/.axon_site/_ro/trn_rl_repo"):
    if os.path.isdir(_p) and _p not in sys.path:
        sys.path.append(_p)

# The axon NTFF profiling hook (antenv.axon_hooks) is absent in this
# container; make sure run_bass_kernel_spmd never takes the trace path.
os.environ["BASS_NEVER_TRACE"] = "1"

import numpy as np
from collections import deque
from contextlib import ExitStack

import concourse.bass as bass
import concourse.tile as tile
from concourse import bacc, mybir
from concourse.bass_utils import run_bass_kernel_spmd

B, C, T, H, W = 2, 64, 8, 48, 48
C2 = 32
P = H * W                      # 2304
N_CORES = 8
S_PER_CORE = (B * T) // N_CORES  # 2 slices per core
QT = P // 128                  # 18 q-tiles of 128
GSZ = 3                        # q-tiles per exp group (3 PSUM banks)
P_CHUNKS = [(0, 512), (512, 512), (1024, 512), (1536, 512), (2048, 256)]
MV = 80                        # val-matmul M: C+1 rows padded to 16n (fp8
                               # DoubleRow ldweights wants 16n columns)
SCALE = 1.0 / np.sqrt(np.float32(C2))
# Schraudolph constants for fp8e5m2: int8(z * 4*log2(e)*SCALE + 60.3).
# HW rounds-to-nearest (measured: HW err matches the rint model to 4
# digits); +0.3 is the center of the flat optimum of the shift scan.
SCH_A = float(4.0 * 1.4426950408889634 * SCALE)
SCH_B = 60.3

# exp-group engine pattern: 2/3 DVE, 1/3 ACT (ACT otherwise only does the
# per-chunk PSUM->SBUF copies; HW DVE ~0.54ns/elem, ACT ~0.9)
EXP_PAT = ("D", "D", "A", "D", "A", "D")

F32 = mybir.dt.float32
BF16 = mybir.dt.bfloat16
I8 = mybir.dt.int8
FP8V = mybir.dt.float8e4       # e4m3 value rows
FP8E = mybir.dt.float8e5       # e5m2 exp bits
DRM = mybir.MatmulPerfMode.DoubleRow
ALU = mybir.AluOpType

_CACHE = {}


def build_nc(repeat=1):
    """Build the per-core Bass program (SPMD: same NEFF on all 8 cores).

    repeat > 1 re-runs the whole computation; used only for timing (the
    extra passes recompute and overwrite the same outputs).
    """
    nc = bacc.Bacc("TRN2", target_bir_lowering=False, debug=False,
                   num_devices=N_CORES)
    # xz: rows 0-63 = x, rows 64-127 = 0 (zero-padded K=128 lhsT)
    xz_d = nc.dram_tensor("xz", [128, S_PER_CORE * P], BF16,
                          kind="ExternalInput").ap()
    # ur: rows 0-63 = u = (phi_w^T theta_w) x; rows 64-127 duplicate u
    # (they only ever multiply the zero weight rows)
    ur_d = nc.dram_tensor("ur", [128, S_PER_CORE * P], BF16,
                          kind="ExternalInput").ap()
    vte_d = nc.dram_tensor("vte", [128, S_PER_CORE * (QT // 2) * 2 * MV],
                           FP8V, kind="ExternalInput").ap()
    y_d = nc.dram_tensor("y", [S_PER_CORE, C + 1, P], F32,
                         kind="ExternalOutput").ap()

    with tile.TileContext(nc) as tc, ExitStack() as ctx:
        ins = ctx.enter_context(tc.tile_pool(name="ins", bufs=2))
        epool = ctx.enter_context(tc.tile_pool(name="epool", bufs=4))
        scp = ctx.enter_context(tc.tile_pool(name="scp", bufs=2, space="PSUM"))
        valp = ctx.enter_context(tc.tile_pool(name="valp", bufs=2,
                                              space="PSUM"))
        epi = ctx.enter_context(tc.tile_pool(name="epi", bufs=2))

        eng = [0]
        cp = [0]
        for r in range(repeat):
            xz_sb = ins.tile([128, S_PER_CORE * P], BF16, tag="xz")
            nc.sync.dma_start(out=xz_sb, in_=xz_d)
            ur_sb = ins.tile([128, S_PER_CORE * P], BF16, tag="ur")
            nc.sync.dma_start(out=ur_sb, in_=ur_d)
            vte_sb = ins.tile([128, S_PER_CORE, QT // 2, 2, MV], FP8V,
                              tag="vte")
            nc.sync.dma_start(out=vte_sb, in_=vte_d.rearrange(
                "p (s g i m) -> p s g i m", s=S_PER_CORE, g=QT // 2, i=2))

            for s in range(S_PER_CORE):
                x0 = s * P
                o_slice = epi.tile([C + 1, P], F32, tag="oslice")
                for (off, w) in P_CHUNKS:
                    val = valp.tile([MV, w], F32, tag="val")

                    def emit_val(e6, k):
                        # val[m, p] += sum_{i,q} vte[q, i, m] * E[q, i, p]
                        # (fp8 DoubleRow: each instr contracts a contiguous
                        # [128, 2, w] pair block of the e6 tile)
                        for j in range(3):
                            pair = k * 3 + j
                            nc.tensor.matmul(
                                out=val,
                                lhsT=vte_sb[:, s, pair, :, :],
                                rhs=e6[:, 2 * j:2 * j + 2, :].bitcast(FP8E),
                                start=(pair == 0),
                                stop=(pair == QT // 2 - 1),
                                perf_mode=DRM,
                            )

                    # software pipeline: the val matmuls of group g-2 are
                    # emitted AFTER the scores of group g, so the PE queue
                    # always holds work that does not depend on the exp of
                    # the group currently on the DVE (the PE engine queue is
                    # strict FIFO - without this, every group serializes
                    # into a PE -> DVE -> PE round-trip). NOTE: carrying the
                    # pending val matmuls across the chunk boundary (so the
                    # per-chunk copy lands between exp ops on the DVE) was
                    # measured 2x SLOWER - it just moves the head-of-line
                    # blocking onto the DVE queue. Keep the drain per-chunk.
                    pend = deque()
                    e6 = None
                    for g in range(QT // GSZ):
                        sc = scp.tile([128, GSZ, w], F32, tag="sc")
                        for j in range(GSZ):
                            qt = g * GSZ + j
                            # scoresT[q, p] = sum_c x[c, q] * u[c, p]
                            # (K=128; xz rows 64-127 are zero)
                            nc.tensor.matmul(
                                out=sc[:, j, :],
                                lhsT=xz_sb[:, x0 + qt * 128:
                                           x0 + (qt + 1) * 128],
                                rhs=ur_sb[:, x0 + off:x0 + off + w],
                                start=True, stop=True,
                            )
                        if len(pend) == 2:
                            emit_val(*pend.popleft())
                        # E = schraudolph-e5m2-exp(sc * SCALE) in int8
                        # bits, alternating DVE / ACT; e6 spans two groups
                        # so the DoubleRow rhs pair blocks are contiguous
                        if g % 2 == 0:
                            e6 = epool.tile([128, 2 * GSZ, w], I8, tag="E",
                                            name="e6")
                        eout = e6[:, GSZ * (g % 2):GSZ * (g % 2) + GSZ, :]
                        if EXP_PAT[eng[0] % len(EXP_PAT)] == "D":
                            nc.vector.tensor_scalar(
                                out=eout, in0=sc, scalar1=SCH_A,
                                scalar2=SCH_B, op0=ALU.mult, op1=ALU.add)
                        else:
                            nc.scalar.activation(
                                out=eout, in_=sc,
                                func=mybir.ActivationFunctionType.Copy,
                                bias=SCH_B, scale=SCH_A)
                        eng[0] += 1
                        if g % 2 == 1:
                            pend.append((e6, g // 2))
                    while pend:
                        emit_val(*pend.popleft())
                    # val -> SBUF on the (otherwise idle) ScalarE: a DVE copy
                    # here would sit between exp ops on the DVE FIFO and
                    # stall them on the PE's last val matmul at every chunk
                    # boundary (same head-of-line pattern as above).
                    if cp[0] % 2 == 0:
                        nc.scalar.copy(out=o_slice[:, off:off + w],
                                       in_=val[:C + 1, :])
                    else:
                        nc.vector.tensor_copy(out=o_slice[:, off:off + w],
                                              in_=val[:C + 1, :])
                    cp[0] += 1
                # output DMA on the qAct ring (inputs ride qSP)
                nc.scalar.dma_start(out=y_d[s], in_=o_slice)

    nc.compile()
    return nc


def host_prep(x_in, theta_w, phi_w, out_w):
    """Per-core input maps: channel projections + device layouts (numpy)."""
    import ml_dtypes
    bf16 = np.dtype(ml_dtypes.bfloat16)
    x_in = np.ascontiguousarray(x_in, dtype=np.float32)
    theta_w = np.asarray(theta_w, dtype=np.float32)
    phi_w = np.asarray(phi_w, dtype=np.float32)
    out_w = np.asarray(out_w, dtype=np.float32)

    x = np.transpose(x_in, (0, 2, 1, 3, 4)).reshape(B, T, C, P)
    G = np.einsum("toc,tod->tcd", phi_w, theta_w)  # [T, C, C]

    in_maps = []
    for k in range(N_CORES):
        xz = np.zeros((128, S_PER_CORE * P), bf16)
        ur = np.empty((128, S_PER_CORE * P), bf16)
        import ml_dtypes as _md
        f8v = np.dtype(_md.float8_e4m3)
        vte = np.zeros((128, S_PER_CORE * (QT // 2) * 2 * MV), f8v)
        vte_v = vte.reshape(128, S_PER_CORE, QT // 2, 2, MV)
        for s in range(S_PER_CORE):
            g = k * S_PER_CORE + s
            b, t = divmod(g, T)
            xslice = x[b, t]                      # [C, P]
            xz[:C, s * P:(s + 1) * P] = xslice
            u = G[t] @ xslice                     # [C, P]
            ur[:C, s * P:(s + 1) * P] = u
            ur[C:, s * P:(s + 1) * P] = u         # junk rows (zero weights)
            v = out_w @ xslice                    # [64, P]
            vt = np.zeros((QT, 128, MV), f8v)
            vt[:, :, :C] = v.T.reshape(QT, 128, C)
            vt[:, :, C] = 1.0                     # softmax-denominator column
            vte_v[:, s] = np.transpose(
                vt.reshape(QT // 2, 2, 128, MV), (2, 0, 1, 3))
        in_maps.append({"xz": xz, "ur": ur, "vte": vte})
    return in_maps


def assemble(results, x_in):
    out = np.empty((B, C, T, H, W), np.float32)
    for k in range(N_CORES):
        y = results[k]["y"]  # [S_PER_CORE, C+1, P]: numerator rows + denom
        for s in range(S_PER_CORE):
            g = k * S_PER_CORE + s
            b, t = divmod(g, T)
            yn = y[s, :C] / y[s, C:C + 1]
            out[b, :, t] = yn.reshape(C, H, W) + x_in[b, :, t]
    return out


def kernel(x_in, theta_w, phi_w, out_w):
    if "nc" not in _CACHE:
        _CACHE["nc"] = build_nc()
    nc = _CACHE["nc"]
    in_maps = host_prep(x_in, theta_w, phi_w, out_w)
    res = run_bass_kernel_spmd(nc, in_maps, core_ids=list(range(N_CORES)))
    return assemble(res.results, np.asarray(x_in, dtype=np.float32))


# revision 14
# speedup vs baseline: 1.1918x; 1.1918x over previous
"""Trainium2 Bass kernel for per-time-slice spatial self-attention + 1x1 conv.

Math per (b, t) slice (16 slices total):
    x      = x_in[b, :, t]          reshaped [C=64, P=2304]
    theta  = theta_w[t] @ x         [32, P]
    phi    = phi_w[t] @ x           [32, P]
    S      = theta.T @ phi / sqrt(32)          [P, P]
    A      = softmax(S, axis=-1)
    f      = x @ A.T  (f[c,p] = sum_q A[p,q] x[c,q])
    out    = out_w @ f + x

Sharding: the 16 slices are independent -> 2 slices per NeuronCore, no
collectives. Host precomputes the cheap channel projections (u, v below)
and packs layouts; the device runs the O(P^2) attention core:

  per p-chunk (4x512 + 256) accumulating in PSUM over 18 q-tiles of 128:
    scoresT[q, p] = sum_c x[c, q] u[c, p]         (PE, K=128 Gram trick:
        u = (phi_w^T theta_w) x host-side, x zero-padded to 128 rows as
        the stationary operand so junk rhs rows are killed by zero
        weights. HW-measured (microbench.py): K=128 matmuls stream at
        ~167ns for N=512 while the K=32 theta/phi form measures
        300-450ns - partial-K matmuls miss the fast stream path.)
    E ~= exp(scoresT / sqrt(32))                  (VectorE, one fused
        tensor_scalar: i16 = int16(s * 128*log2e*scale + 16256); the i16
        bit pattern bitcast as bf16 IS the Schraudolph base-2 exp
        approximation; the softmax normalization cancels the bulk of the
        correlated approximation error: end-to-end max rel err ~2.4e-3.)
    val[m, p] += vte[q, m]^T E[q, p]              (PE, m: 64 v-channels + ones
                                                   column -> softmax denom)
  epilogue: one ACT copy val -> SBUF per chunk; one [65, 2304] DMA per
  slice (output DMAs ride the qAct HWDGE ring, inputs ride qSP, so the
  two streams don't serialize). The final normalization (row 64 divide)
  and the +x residual are done on the host after the gather.

exp skips max-subtraction (scores ~ N(0,1), max |s| ~ 12.6; safe in this
fixed-point window: t = 128*z + 16256 stays within int16 for |z| < 127).
"""

import os
import sys

for _p in ("/opt/trn_rl_repo", "/root/.axon_site/_ro/trn_rl_repo"):
    if os.path.isdir(_p) and _p not in sys.path:
        sys.path.append(_p)

# The axon NTFF profiling hook (antenv.axon_hooks) is absent in this
# container; make sure run_bass_kernel_spmd never takes the trace path.
os.environ["BASS_NEVER_TRACE"] = "1"

import numpy as np
from collections import deque
from contextlib import ExitStack

import concourse.bass as bass
import concourse.tile as tile
from concourse import bacc, mybir
from concourse.bass_utils import run_bass_kernel_spmd

B, C, T, H, W = 2, 64, 8, 48, 48
C2 = 32
P = H * W                      # 2304
N_CORES = 8
S_PER_CORE = (B * T) // N_CORES  # 2 slices per core
QT = P // 128                  # 18 q-tiles of 128
GSZ = 3                        # q-tiles per exp group (3 PSUM banks)
P_CHUNKS = [(0, 512), (512, 512), (1024, 512), (1536, 512), (2048, 256)]
MV = 80                        # val-matmul M: C+1 rows padded to 16n (fp8
                               # DoubleRow ldweights wants 16n columns)
SCALE = 1.0 / np.sqrt(np.float32(C2))
# Schraudolph constants for fp8e5m2: int8(z * 4*log2(e)*SCALE + 60.3).
# HW rounds-to-nearest (measured: HW err matches the rint model to 4
# digits); +0.3 is the center of the flat optimum of the shift scan.
SCH_A = float(4.0 * 1.4426950408889634 * SCALE)
SCH_B = 60.3

# exp-group engine pattern: 2/3 DVE, 1/3 ACT (ACT otherwise only does the
# per-chunk PSUM->SBUF copies; HW DVE ~0.54ns/elem, ACT ~0.9)
EXP_PAT = ("D", "D", "A")

F32 = mybir.dt.float32
BF16 = mybir.dt.bfloat16
I8 = mybir.dt.int8
FP8V = mybir.dt.float8e4       # e4m3 value rows
FP8E = mybir.dt.float8e5       # e5m2 exp bits
DRM = mybir.MatmulPerfMode.DoubleRow
ALU = mybir.AluOpType

_CACHE = {}


def build_nc(repeat=1):
    """Build the per-core Bass program (SPMD: same NEFF on all 8 cores).

    repeat > 1 re-runs the whole computation; used only for timing (the
    extra passes recompute and overwrite the same outputs).
    """
    nc = bacc.Bacc("TRN2", target_bir_lowering=False, debug=False,
                   num_devices=N_CORES)
    # xz: rows 0-63 = x, rows 64-127 = 0 (zero-padded K=128 lhsT)
    xz_d = nc.dram_tensor("xz", [128, S_PER_CORE * P], BF16,
                          kind="ExternalInput").ap()
    # ur: rows 0-63 = u = (phi_w^T theta_w) x; rows 64-127 duplicate u
    # (they only ever multiply the zero weight rows)
    ur_d = nc.dram_tensor("ur", [128, S_PER_CORE * P], BF16,
                          kind="ExternalInput").ap()
    vte_d = nc.dram_tensor("vte", [128, S_PER_CORE * (QT // 2) * 2 * MV],
                           FP8V, kind="ExternalInput").ap()
    y_d = nc.dram_tensor("y", [S_PER_CORE, C + 1, P], F32,
                         kind="ExternalOutput").ap()

    with tile.TileContext(nc) as tc, ExitStack() as ctx:
        ins = ctx.enter_context(tc.tile_pool(name="ins", bufs=2))
        epool = ctx.enter_context(tc.tile_pool(name="epool", bufs=4))
        scp = ctx.enter_context(tc.tile_pool(name="scp", bufs=2, space="PSUM"))
        valp = ctx.enter_context(tc.tile_pool(name="valp", bufs=2,
                                              space="PSUM"))
        epi = ctx.enter_context(tc.tile_pool(name="epi", bufs=2))

        eng = [0]
        for r in range(repeat):
            xz_sb = ins.tile([128, S_PER_CORE * P], BF16, tag="xz")
            nc.sync.dma_start(out=xz_sb, in_=xz_d)
            ur_sb = ins.tile([128, S_PER_CORE * P], BF16, tag="ur")
            nc.sync.dma_start(out=ur_sb, in_=ur_d)
            vte_sb = ins.tile([128, S_PER_CORE, QT // 2, 2, MV], FP8V,
                              tag="vte")
            nc.sync.dma_start(out=vte_sb, in_=vte_d.rearrange(
                "p (s g i m) -> p s g i m", s=S_PER_CORE, g=QT // 2, i=2))

            for s in range(S_PER_CORE):
                x0 = s * P
                o_slice = epi.tile([C + 1, P], F32, tag="oslice")
                for (off, w) in P_CHUNKS:
                    val = valp.tile([MV, w], F32, tag="val")

                    def emit_val(e6, k):
                        # val[m, p] += sum_{i,q} vte[q, i, m] * E[q, i, p]
                        # (fp8 DoubleRow: each instr contracts a contiguous
                        # [128, 2, w] pair block of the e6 tile)
                        for j in range(3):
                            pair = k * 3 + j
                            nc.tensor.matmul(
                                out=val,
                                lhsT=vte_sb[:, s, pair, :, :],
                                rhs=e6[:, 2 * j:2 * j + 2, :].bitcast(FP8E),
                                start=(pair == 0),
                                stop=(pair == QT // 2 - 1),
                                perf_mode=DRM,
                            )

                    # software pipeline: the val matmuls of group g-2 are
                    # emitted AFTER the scores of group g, so the PE queue
                    # always holds work that does not depend on the exp of
                    # the group currently on the DVE (the PE engine queue is
                    # strict FIFO - without this, every group serializes
                    # into a PE -> DVE -> PE round-trip). NOTE: carrying the
                    # pending val matmuls across the chunk boundary (so the
                    # per-chunk copy lands between exp ops on the DVE) was
                    # measured 2x SLOWER - it just moves the head-of-line
                    # blocking onto the DVE queue. Keep the drain per-chunk.
                    pend = deque()
                    e6 = None
                    for g in range(QT // GSZ):
                        sc = scp.tile([128, GSZ, w], F32, tag="sc")
                        for j in range(GSZ):
                            qt = g * GSZ + j
                            # scoresT[q, p] = sum_c x[c, q] * u[c, p]
                            # (K=128; xz rows 64-127 are zero)
                            nc.tensor.matmul(
                                out=sc[:, j, :],
                                lhsT=xz_sb[:, x0 + qt * 128:
                                           x0 + (qt + 1) * 128],
                                rhs=ur_sb[:, x0 + off:x0 + off + w],
                                start=True, stop=True,
                            )
                        if len(pend) == 2:
                            emit_val(*pend.popleft())
                        # E = schraudolph-e5m2-exp(sc * SCALE) in int8
                        # bits, alternating DVE / ACT; e6 spans two groups
                        # so the DoubleRow rhs pair blocks are contiguous
                        if g % 2 == 0:
                            e6 = epool.tile([128, 2 * GSZ, w], I8, tag="E",
                                            name="e6")
                        eout = e6[:, GSZ * (g % 2):GSZ * (g % 2) + GSZ, :]
                        if EXP_PAT[eng[0] % len(EXP_PAT)] == "D":
                            nc.vector.tensor_scalar(
                                out=eout, in0=sc, scalar1=SCH_A,
                                scalar2=SCH_B, op0=ALU.mult, op1=ALU.add)
                        else:
                            nc.scalar.activation(
                                out=eout, in_=sc,
                                func=mybir.ActivationFunctionType.Copy,
                                bias=SCH_B, scale=SCH_A)
                        eng[0] += 1
                        if g % 2 == 1:
                            pend.append((e6, g // 2))
                    while pend:
                        emit_val(*pend.popleft())
                    # val -> SBUF on the (otherwise idle) ScalarE: a DVE copy
                    # here would sit between exp ops on the DVE FIFO and
                    # stall them on the PE's last val matmul at every chunk
                    # boundary (same head-of-line pattern as above).
                    nc.scalar.copy(out=o_slice[:, off:off + w],
                                   in_=val[:C + 1, :])
                # output DMA on the qAct ring (inputs ride qSP)
                nc.scalar.dma_start(out=y_d[s], in_=o_slice)

    nc.compile()
    return nc


def host_prep(x_in, theta_w, phi_w, out_w):
    """Per-core input maps: channel projections + device layouts (numpy)."""
    import ml_dtypes
    bf16 = np.dtype(ml_dtypes.bfloat16)
    x_in = np.ascontiguousarray(x_in, dtype=np.float32)
    theta_w = np.asarray(theta_w, dtype=np.float32)
    phi_w = np.asarray(phi_w, dtype=np.float32)
    out_w = np.asarray(out_w, dtype=np.float32)

    x = np.transpose(x_in, (0, 2, 1, 3, 4)).reshape(B, T, C, P)
    G = np.einsum("toc,tod->tcd", phi_w, theta_w)  # [T, C, C]

    in_maps = []
    for k in range(N_CORES):
        xz = np.zeros((128, S_PER_CORE * P), bf16)
        ur = np.empty((128, S_PER_CORE * P), bf16)
        import ml_dtypes as _md
        f8v = np.dtype(_md.float8_e4m3)
        vte = np.zeros((128, S_PER_CORE * (QT // 2) * 2 * MV), f8v)
        vte_v = vte.reshape(128, S_PER_CORE, QT // 2, 2, MV)
        for s in range(S_PER_CORE):
            g = k * S_PER_CORE + s
            b, t = divmod(g, T)
            xslice = x[b, t]                      # [C, P]
            xz[:C, s * P:(s + 1) * P] = xslice
            u = G[t] @ xslice                     # [C, P]
            ur[:C, s * P:(s + 1) * P] = u
            ur[C:, s * P:(s + 1) * P] = u         # junk rows (zero weights)
            v = out_w @ xslice                    # [64, P]
            vt = np.zeros((QT, 128, MV), f8v)
            vt[:, :, :C] = v.T.reshape(QT, 128, C)
            vt[:, :, C] = 1.0                     # softmax-denominator column
            vte_v[:, s] = np.transpose(
                vt.reshape(QT // 2, 2, 128, MV), (2, 0, 1, 3))
        in_maps.append({"xz": xz, "ur": ur, "vte": vte})
    return in_maps


def assemble(results, x_in):
    out = np.empty((B, C, T, H, W), np.float32)
    for k in range(N_CORES):
        y = results[k]["y"]  # [S_PER_CORE, C+1, P]: numerator rows + denom
        for s in range(S_PER_CORE):
            g = k * S_PER_CORE + s
            b, t = divmod(g, T)
            yn = y[s, :C] / y[s, C:C + 1]
            out[b, :, t] = yn.reshape(C, H, W) + x_in[b, :, t]
    return out


def kernel(x_in, theta_w, phi_w, out_w):
    if "nc" not in _CACHE:
        _CACHE["nc"] = build_nc()
    nc = _CACHE["nc"]
    in_maps = host_prep(x_in, theta_w, phi_w, out_w)
    res = run_bass_kernel_spmd(nc, in_maps, core_ids=list(range(N_CORES)))
    return assemble(res.results, np.asarray(x_in, dtype=np.float32))
